# revision 9
# baseline (speedup 1.0000x reference)
"""Trainium2 Bass kernel for a 4-layer LIF spiking net scanned over T=32 steps.

Strategy (data-parallel, 8 cores):
  - Shard batch B=2048 -> 256 per core; weights replicated.
  - On-device layout is feature-on-partitions [h, b]: every matmul's
    stationary operand is a static weight tile, spikes are the moving
    operand, so the whole recurrence needs zero on-device transposes.
  - Per layer/step: PE does only the W matmuls (PSUM = W @ s, fp32).
    ScalarE copies PSUM->SBUF fusing the per-partition bias
    (c = Identity(psum + b[h])). VectorE does the membrane update in two
    fused scalar_tensor_tensor ops: u = beta*m_prev + c, then in-place
    m = (-thr)*s_prev + u. GpSimdE computes s = (m > thr).
    (Note reset_t = H(m_{t-1}-thr) == s_{t-1}, so no extra heaviside.)
  - c/m/s are written into [psz, G*BC] staging tiles (G=4 timesteps) and
    DMA'd as single fully-contiguous 512KB blocks into [T/G, H, G, B]
    scratch layout; host unpacks to [T, B, H].
"""

import sys

if "/opt/trn_rl_repo" not in sys.path:
    sys.path.insert(0, "/opt/trn_rl_repo")

import numpy as np

T, B, D, H, D4 = 32, 2048, 48, 256, 10
NCORES = 8
BC = B // NCORES  # 256 batch rows per core
P = 128
G = 4  # timesteps per output staging group
NG = T // G


def _build(betas, thrs):
    """Build the SPMD Bass program (identical on all cores)."""
    import concourse.mybir as mybir
    from concourse import bacc
    from concourse.tile import TileContext

    f32 = mybir.dt.float32
    Alu = mybir.AluOpType

    # Bacc (not raw Bass): its compile() runs move_matmul_waits_to_ldweights /
    # generate_event_semaphores, which walrus requires (1 sync-wait per inst).
    nc = bacc.Bacc(target_bir_lowering=False)

    # ---- DRAM I/O ----
    xT_d = nc.dram_tensor("xT", [D, T * BC], f32, kind="ExternalInput")
    w1_d = nc.dram_tensor("w1t", [D, H], f32, kind="ExternalInput")
    w2_d = nc.dram_tensor("w2t", [H, H], f32, kind="ExternalInput")
    w3_d = nc.dram_tensor("w3t", [H, H], f32, kind="ExternalInput")
    w4_d = nc.dram_tensor("w4t", [H, D4], f32, kind="ExternalInput")
    b_d = [
        nc.dram_tensor("b1", [H, 1], f32, kind="ExternalInput"),
        nc.dram_tensor("b2", [H, 1], f32, kind="ExternalInput"),
        nc.dram_tensor("b3", [H, 1], f32, kind="ExternalInput"),
        nc.dram_tensor("b4", [D4, 1], f32, kind="ExternalInput"),
    ]
    # outputs in [T/G, hl, G, BC] per-core layout
    so_d, mo_d, co_d = [], [], []
    for l in range(4):
        hl = H if l < 3 else D4
        so_d.append(nc.dram_tensor(f"s{l + 1}o", [NG, hl, G, BC], f32, kind="ExternalOutput"))
        mo_d.append(nc.dram_tensor(f"m{l + 1}o", [NG, hl, G, BC], f32, kind="ExternalOutput"))
        co_d.append(nc.dram_tensor(f"c{l + 1}o", [NG, hl, G, BC], f32, kind="ExternalOutput"))

    ntiles = [2, 2, 2, 1]
    psz = [P, P, P, D4]

    with TileContext(nc) as tc:
        with (
            tc.tile_pool(name="const", bufs=1) as cpool,
            tc.tile_pool(name="xin", bufs=2) as xpool,
            tc.tile_pool(name="stage", bufs=2) as spool,
            tc.tile_pool(name="psum", bufs=8, space="PSUM") as ppool,
        ):
            # ---- load constants ----
            w1_sb = cpool.tile([D, H], f32, name="w1_sb")
            nc.sync.dma_start(w1_sb[:], w1_d[:])
            w2_sb = [cpool.tile([P, H], f32, name=f"w2_sb{j}") for j in range(2)]
            w3_sb = [cpool.tile([P, H], f32, name=f"w3_sb{j}") for j in range(2)]
            w4_sb = [cpool.tile([P, D4], f32, name=f"w4_sb{j}") for j in range(2)]
            for j in range(2):
                nc.sync.dma_start(w2_sb[j][:], w2_d[j * P : (j + 1) * P, :])
                nc.sync.dma_start(w3_sb[j][:], w3_d[j * P : (j + 1) * P, :])
                nc.sync.dma_start(w4_sb[j][:], w4_d[j * P : (j + 1) * P, :])
            wk = [[w1_sb], w2_sb, w3_sb, w4_sb]  # k-tiles of lhsT per layer
            # per-partition bias columns, sliced per output tile
            b_sb = []
            for l in range(4):
                cols = []
                for tau in range(ntiles[l]):
                    t_ = cpool.tile([psz[l], 1], f32, name=f"b_sb{l}_{tau}")
                    nc.sync.dma_start(t_[:], b_d[l][tau * psz[l] : (tau + 1) * psz[l], :])
                    cols.append(t_)
                b_sb.append(cols)

            # ---- state init (zero tiles) ----
            m_prev = {}
            s_prev = {}
            for l in range(4):
                for tau in range(ntiles[l]):
                    mt = cpool.tile([psz[l], BC], f32, name=f"m0_{l}_{tau}")
                    nc.vector.memset(mt[:], 0.0)
                    m_prev[(l, tau)] = mt
                    st = cpool.tile([psz[l], BC], f32, name=f"s0_{l}_{tau}")
                    nc.vector.memset(st[:], 0.0)
                    s_prev[(l, tau)] = st

            # ---- time loop: G-step groups, fully unrolled ----
            for tg in range(NG):
                # input slab for this group: [48, G*BC]
                xg = xpool.tile([D, G * BC], f32, tag="xg", name=f"xg{tg}")
                nc.sync.dma_start(xg[:], xT_d[:, tg * G * BC : (tg + 1) * G * BC])

                # staging tiles for this group
                cst, mst, sst = {}, {}, {}
                for l in range(4):
                    for tau in range(ntiles[l]):
                        cst[(l, tau)] = spool.tile(
                            [psz[l], G * BC], f32, tag=f"c{l}_{tau}", name=f"c{l}_{tau}_{tg}"
                        )
                        mst[(l, tau)] = spool.tile(
                            [psz[l], G * BC], f32, tag=f"m{l}_{tau}", name=f"m{l}_{tau}_{tg}"
                        )
                        sst[(l, tau)] = spool.tile(
                            [psz[l], G * BC], f32, tag=f"s{l}_{tau}", name=f"s{l}_{tau}_{tg}"
                        )

                for g in range(G):
                    gs = slice(g * BC, (g + 1) * BC)
                    rhs_tiles = [xg[:, gs]]  # layer-1 moving operand
                    for l in range(4):
                        new_s = []
                        for tau in range(ntiles[l]):
                            sl = slice(tau * psz[l], (tau + 1) * psz[l])
                            ps = ppool.tile([psz[l], BC], f32, tag="ps", name=f"ps{l}_{tau}")
                            for j, rhs in enumerate(rhs_tiles):
                                nc.tensor.matmul(
                                    ps[:],
                                    wk[l][j][:, sl],
                                    rhs,
                                    start=(j == 0),
                                    stop=(j == len(rhs_tiles) - 1),
                                )
                            c = cst[(l, tau)][:, gs]
                            m = mst[(l, tau)][:, gs]
                            s = sst[(l, tau)][:, gs]
                            # c = psum + bias[h]  (ACT, PSUM->SBUF with fused bias)
                            nc.scalar.add(c, ps[:], b_sb[l][tau][:])
                            # u = beta*m_prev + c ; m = (-thr)*s_prev + u (in place)
                            nc.vector.scalar_tensor_tensor(
                                m, m_prev[(l, tau)][:], betas[l], c, Alu.mult, Alu.add
                            )
                            nc.vector.scalar_tensor_tensor(
                                m, s_prev[(l, tau)][:], -thrs[l], m, Alu.mult, Alu.add
                            )
                            # s = (m > thr)  (GpSimd; 1-input op runs ~line rate there)
                            nc.gpsimd.tensor_scalar(s, m, thrs[l], None, Alu.is_gt)
                            m_prev[(l, tau)] = m
                            s_prev[(l, tau)] = s
                            new_s.append(s)
                        rhs_tiles = new_s

                # one contiguous 512KB DMA per (tensor, htile) per group
                for l in range(4):
                    hl = H if l < 3 else D4
                    for tau in range(ntiles[l]):
                        sl = slice(tau * psz[l], (tau + 1) * psz[l])
                        nc.sync.dma_start(
                            co_d[l][tg, sl, :, :], cst[(l, tau)].rearrange("p (g b) -> p g b", g=G)
                        )
                        nc.sync.dma_start(
                            mo_d[l][tg, sl, :, :], mst[(l, tau)].rearrange("p (g b) -> p g b", g=G)
                        )
                        nc.sync.dma_start(
                            so_d[l][tg, sl, :, :], sst[(l, tau)].rearrange("p (g b) -> p g b", g=G)
                        )

    nc.compile()
    return nc


LAST = None  # last BassKernelResults (for test harness: exec_time_ns, trace)
NC = None


def kernel(**inputs):
    import os

    from concourse.bass_utils import run_bass_kernel_spmd

    x = np.asarray(inputs["x"], np.float32)
    Ws = [np.asarray(inputs[f"W{i}"], np.float32) for i in (1, 2, 3, 4)]
    bs = [np.asarray(inputs[f"b{i}"], np.float32) for i in (1, 2, 3, 4)]
    betas = [float(np.clip(np.float32(inputs[f"beta{i}"]), 0.0, 1.0)) for i in (1, 2, 3, 4)]
    thrs = [float(np.float32(inputs[f"thr{i}"])) for i in (1, 2, 3, 4)]

    nc = _build(betas, thrs)
    global NC
    NC = nc

    shared = {
        "w1t": np.ascontiguousarray(Ws[0].T),
        "w2t": np.ascontiguousarray(Ws[1].T),
        "w3t": np.ascontiguousarray(Ws[2].T),
        "w4t": np.ascontiguousarray(Ws[3].T),
        "b1": np.ascontiguousarray(bs[0].reshape(H, 1)),
        "b2": np.ascontiguousarray(bs[1].reshape(H, 1)),
        "b3": np.ascontiguousarray(bs[2].reshape(H, 1)),
        "b4": np.ascontiguousarray(bs[3].reshape(D4, 1)),
    }
    in_maps = []
    for c in range(NCORES):
        xc = x[c * BC : (c + 1) * BC]  # [BC, T, D]
        xT = np.ascontiguousarray(xc.transpose(2, 1, 0).reshape(D, T * BC))
        m = dict(shared)
        m["xT"] = xT
        in_maps.append(m)

    kwargs = {}
    if os.environ.get("KTRACE"):
        kwargs["trace"] = True
        if os.environ.get("KTRACE_DIR"):
            kwargs["tmpdir"] = os.environ["KTRACE_DIR"]
    res = run_bass_kernel_spmd(nc, in_maps, core_ids=list(range(NCORES)), **kwargs)
    global LAST
    LAST = res
    results = res.results

    outs = []
    for kind in ("s", "m", "c"):
        for l in range(4):
            hl = H if l < 3 else D4
            full = np.empty((T, B, hl), np.float32)
            for c in range(NCORES):
                dev = results[c][f"{kind}{l + 1}o"]  # [NG, hl, G, BC]
                full[:, c * BC : (c + 1) * BC, :] = dev.transpose(0, 2, 3, 1).reshape(T, BC, hl)
            outs.append(full)
    # reference order: (s1..s4, m1..m4, c1..c4)
    return tuple(outs)


if __name__ == "__main__":
    pass


# revision 12
# speedup vs baseline: 3.2236x; 3.2236x over previous
"""Trainium2 Bass kernel for a 4-layer LIF spiking net scanned over T=32 steps.

Strategy (data-parallel, 8 cores):
  - Shard batch B=2048 -> 256 per core; weights replicated.
  - On-device layout is feature-on-partitions [h, b]: every matmul's
    stationary operand is a static weight tile, spikes are the moving
    operand, so the whole recurrence needs zero on-device transposes.
  - Per layer/step: PE does only the W matmuls (PSUM = W @ s, fp32).
    ScalarE copies PSUM->SBUF fusing the per-partition bias
    (c = Identity(psum + b[h])). VectorE does the membrane update in two
    fused scalar_tensor_tensor ops: u = beta*m_prev + c, then in-place
    m = (-thr)*s_prev + u. GpSimdE computes s = (m > thr).
    (Note reset_t = H(m_{t-1}-thr) == s_{t-1}, so no extra heaviside.)
  - c/m/s are written into [psz, G*BC] staging tiles (G=4 timesteps) and
    DMA'd as single fully-contiguous 512KB blocks into [T/G, H, G, B]
    scratch layout; host unpacks to [T, B, H].
"""

import sys

if "/opt/trn_rl_repo" not in sys.path:
    sys.path.insert(0, "/opt/trn_rl_repo")

import numpy as np

T, B, D, H, D4 = 32, 2048, 48, 256, 10
NCORES = 8
BC = B // NCORES  # 256 batch rows per core
P = 128
G = 4  # timesteps per output staging group
NG = T // G


def _build(betas, thrs):
    """Build the SPMD Bass program (identical on all cores)."""
    import concourse.mybir as mybir
    from concourse import bacc
    from concourse.tile import TileContext

    f32 = mybir.dt.float32
    Alu = mybir.AluOpType

    # Bacc (not raw Bass): its compile() runs move_matmul_waits_to_ldweights /
    # generate_event_semaphores, which walrus requires (1 sync-wait per inst).
    nc = bacc.Bacc(target_bir_lowering=False)

    # ---- DRAM I/O ----
    xT_d = nc.dram_tensor("xT", [D, T * BC], f32, kind="ExternalInput")
    w1_d = nc.dram_tensor("w1t", [D, H], f32, kind="ExternalInput")
    w2_d = nc.dram_tensor("w2t", [H, H], f32, kind="ExternalInput")
    w3_d = nc.dram_tensor("w3t", [H, H], f32, kind="ExternalInput")
    w4_d = nc.dram_tensor("w4t", [H, D4], f32, kind="ExternalInput")
    b_d = [
        nc.dram_tensor("b1", [H, 1], f32, kind="ExternalInput"),
        nc.dram_tensor("b2", [H, 1], f32, kind="ExternalInput"),
        nc.dram_tensor("b3", [H, 1], f32, kind="ExternalInput"),
        nc.dram_tensor("b4", [D4, 1], f32, kind="ExternalInput"),
    ]
    # outputs in [T/G, hl, G, BC] per-core layout
    so_d, mo_d, co_d = [], [], []
    for l in range(4):
        hl = H if l < 3 else D4
        so_d.append(nc.dram_tensor(f"s{l + 1}o", [NG, hl, G, BC], f32, kind="ExternalOutput"))
        mo_d.append(nc.dram_tensor(f"m{l + 1}o", [NG, hl, G, BC], f32, kind="ExternalOutput"))
        co_d.append(nc.dram_tensor(f"c{l + 1}o", [NG, hl, G, BC], f32, kind="ExternalOutput"))

    ntiles = [2, 2, 2, 1]
    psz = [P, P, P, D4]

    with TileContext(nc) as tc:
        with (
            tc.tile_pool(name="const", bufs=1) as cpool,
            tc.tile_pool(name="xin", bufs=2) as xpool,
            tc.tile_pool(name="stage", bufs=2) as spool,
            tc.tile_pool(name="psum", bufs=8, space="PSUM") as ppool,
        ):
            # ---- load constants ----
            w1_sb = cpool.tile([D, H], f32, name="w1_sb")
            nc.sync.dma_start(w1_sb[:], w1_d[:])
            w2_sb = [cpool.tile([P, H], f32, name=f"w2_sb{j}") for j in range(2)]
            w3_sb = [cpool.tile([P, H], f32, name=f"w3_sb{j}") for j in range(2)]
            w4_sb = [cpool.tile([P, D4], f32, name=f"w4_sb{j}") for j in range(2)]
            for j in range(2):
                nc.sync.dma_start(w2_sb[j][:], w2_d[j * P : (j + 1) * P, :])
                nc.sync.dma_start(w3_sb[j][:], w3_d[j * P : (j + 1) * P, :])
                nc.sync.dma_start(w4_sb[j][:], w4_d[j * P : (j + 1) * P, :])
            wk = [[w1_sb], w2_sb, w3_sb, w4_sb]  # k-tiles of lhsT per layer
            # per-partition bias columns, sliced per output tile
            b_sb = []
            for l in range(4):
                cols = []
                for tau in range(ntiles[l]):
                    t_ = cpool.tile([psz[l], 1], f32, name=f"b_sb{l}_{tau}")
                    nc.sync.dma_start(t_[:], b_d[l][tau * psz[l] : (tau + 1) * psz[l], :])
                    cols.append(t_)
                b_sb.append(cols)

            # ---- state: None until t=0 writes it (m(0) = c(0) exactly) ----
            m_prev = {}
            s_prev = {}
            for l in range(4):
                for tau in range(ntiles[l]):
                    m_prev[(l, tau)] = None
                    s_prev[(l, tau)] = None

            # ---- time loop: G-step groups, fully unrolled ----
            for tg in range(NG):
                # input slab for this group: [48, G*BC]
                xg = xpool.tile([D, G * BC], f32, tag="xg", name=f"xg{tg}")
                nc.sync.dma_start(xg[:], xT_d[:, tg * G * BC : (tg + 1) * G * BC])

                # staging tiles for this group
                cst, mst, sst = {}, {}, {}
                for l in range(4):
                    for tau in range(ntiles[l]):
                        cst[(l, tau)] = spool.tile(
                            [psz[l], G * BC], f32, tag=f"c{l}_{tau}", name=f"c{l}_{tau}_{tg}"
                        )
                        mst[(l, tau)] = spool.tile(
                            [psz[l], G * BC], f32, tag=f"m{l}_{tau}", name=f"m{l}_{tau}_{tg}"
                        )
                        sst[(l, tau)] = spool.tile(
                            [psz[l], G * BC], f32, tag=f"s{l}_{tau}", name=f"s{l}_{tau}_{tg}"
                        )

                # layer-1 matmuls depend only on x: batch 2 timesteps (N=512)
                # per call and copy out c1 (with bias) for the pair at once.
                for gp in range(0, G, 2):
                    gps = slice(gp * BC, (gp + 2) * BC)
                    for tau in range(ntiles[0]):
                        sl = slice(tau * psz[0], (tau + 1) * psz[0])
                        ps = ppool.tile([psz[0], 2 * BC], f32, tag="ps", name=f"ps1w_{tau}")
                        nc.tensor.matmul(ps[:], wk[0][0][:, sl], xg[:, gps], start=True, stop=True)
                        nc.scalar.add(cst[(0, tau)][:, gps], ps[:], b_sb[0][tau][:])

                for g in range(G):
                    gs = slice(g * BC, (g + 1) * BC)
                    rhs_tiles = None  # layer-1 c is precomputed above
                    for l in range(4):
                        new_s = []
                        for tau in range(ntiles[l]):
                            sl = slice(tau * psz[l], (tau + 1) * psz[l])
                            c = cst[(l, tau)][:, gs]
                            m = mst[(l, tau)][:, gs]
                            s = sst[(l, tau)][:, gs]
                            if l > 0:
                                ps = ppool.tile([psz[l], BC], f32, tag="ps", name=f"ps{l}_{tau}")
                                for j, rhs in enumerate(rhs_tiles):
                                    nc.tensor.matmul(
                                        ps[:],
                                        wk[l][j][:, sl],
                                        rhs,
                                        start=(j == 0),
                                        stop=(j == len(rhs_tiles) - 1),
                                    )
                                # c = psum + bias[h]  (ACT, PSUM->SBUF + fused bias)
                                nc.scalar.add(c, ps[:], b_sb[l][tau][:])
                            if m_prev[(l, tau)] is None:
                                # t=0: m = beta*0 + c - thr*0 = c
                                nc.vector.tensor_copy(m, c)
                            else:
                                # u = beta*m_prev + c ; m = (-thr)*s_prev + u
                                u = spool.tile(
                                    [psz[l], BC], f32, tag=f"u{l}_{tau}", bufs=2, name=f"u{l}_{tau}"
                                )
                                nc.vector.scalar_tensor_tensor(
                                    u[:], m_prev[(l, tau)][:], betas[l], c, Alu.mult, Alu.add
                                )
                                nc.vector.scalar_tensor_tensor(
                                    m, s_prev[(l, tau)][:], -thrs[l], u[:], Alu.mult, Alu.add
                                )
                            # s = (m > thr)
                            nc.vector.tensor_scalar(s, m, thrs[l], None, Alu.is_gt)
                            m_prev[(l, tau)] = m
                            s_prev[(l, tau)] = s
                            new_s.append(s)
                        rhs_tiles = new_s

                # one contiguous 512KB DMA per (tensor, htile) per group
                for l in range(4):
                    hl = H if l < 3 else D4
                    for tau in range(ntiles[l]):
                        sl = slice(tau * psz[l], (tau + 1) * psz[l])
                        nc.sync.dma_start(
                            co_d[l][tg, sl, :, :], cst[(l, tau)].rearrange("p (g b) -> p g b", g=G)
                        )
                        nc.sync.dma_start(
                            mo_d[l][tg, sl, :, :], mst[(l, tau)].rearrange("p (g b) -> p g b", g=G)
                        )
                        nc.sync.dma_start(
                            so_d[l][tg, sl, :, :], sst[(l, tau)].rearrange("p (g b) -> p g b", g=G)
                        )

    nc.compile()
    return nc


LAST = None  # last BassKernelResults (for test harness: exec_time_ns, trace)
NC = None


def kernel(**inputs):
    import os

    from concourse.bass_utils import run_bass_kernel_spmd

    x = np.asarray(inputs["x"], np.float32)
    Ws = [np.asarray(inputs[f"W{i}"], np.float32) for i in (1, 2, 3, 4)]
    bs = [np.asarray(inputs[f"b{i}"], np.float32) for i in (1, 2, 3, 4)]
    betas = [float(np.clip(np.float32(inputs[f"beta{i}"]), 0.0, 1.0)) for i in (1, 2, 3, 4)]
    thrs = [float(np.float32(inputs[f"thr{i}"])) for i in (1, 2, 3, 4)]

    nc = _build(betas, thrs)
    global NC
    NC = nc

    shared = {
        "w1t": np.ascontiguousarray(Ws[0].T),
        "w2t": np.ascontiguousarray(Ws[1].T),
        "w3t": np.ascontiguousarray(Ws[2].T),
        "w4t": np.ascontiguousarray(Ws[3].T),
        "b1": np.ascontiguousarray(bs[0].reshape(H, 1)),
        "b2": np.ascontiguousarray(bs[1].reshape(H, 1)),
        "b3": np.ascontiguousarray(bs[2].reshape(H, 1)),
        "b4": np.ascontiguousarray(bs[3].reshape(D4, 1)),
    }
    in_maps = []
    for c in range(NCORES):
        xc = x[c * BC : (c + 1) * BC]  # [BC, T, D]
        xT = np.ascontiguousarray(xc.transpose(2, 1, 0).reshape(D, T * BC))
        m = dict(shared)
        m["xT"] = xT
        in_maps.append(m)

    kwargs = {}
    if os.environ.get("KTRACE"):
        kwargs["trace"] = True
        if os.environ.get("KTRACE_DIR"):
            kwargs["tmpdir"] = os.environ["KTRACE_DIR"]
    res = run_bass_kernel_spmd(nc, in_maps, core_ids=list(range(NCORES)), **kwargs)
    global LAST
    LAST = res
    results = res.results

    outs = []
    for kind in ("s", "m", "c"):
        for l in range(4):
            hl = H if l < 3 else D4
            full = np.empty((T, B, hl), np.float32)
            for c in range(NCORES):
                dev = results[c][f"{kind}{l + 1}o"]  # [NG, hl, G, BC]
                full[:, c * BC : (c + 1) * BC, :] = dev.transpose(0, 2, 3, 1).reshape(T, BC, hl)
            outs.append(full)
    # reference order: (s1..s4, m1..m4, c1..c4)
    return tuple(outs)


if __name__ == "__main__":
    pass
